# revision 2
# baseline (speedup 1.0000x reference)
"""HeterogeneousKANLayer forward on 8 Trainium2 NeuronCores.

Math (reference):
  xn    = tanh(x)                                  [B, I]
  base  = silu(xn)                                 [B, I]
  basis = exp(-((xn - c_j)/w)^2), c_j evenly spaced on [-1,1], w = 2/(C-1)
  out[b,o] = sum_{i,c} basis[b,i,c]*coef[i,o,c]*scale_sp[o,i]
           + sum_i base[b,i]*scale_base[o,i]

Kernel strategy (data-parallel over batch, 8 cores x 512 rows):
  Everything is one [512b, 5632k] @ [5632k, 512o] matmul per core, where
  k = (center, i) channels plus one silu channel group. Host folds
  scale_sp into coef and appends scale_base^T as the last 4 k-tiles.
  Gaussian per center j is computed as u * exp(a_j*xn + b_j) with
  u = exp(-xn^2/w^2): one ACT exp + one DVE multiply per center
  (ACT ~25us, DVE ~25us, PE ~37.5us bf16 - PE-bound at the fp32r/bf16
  roofline for this shard).
"""

import sys
import types

import numpy as np
import ml_dtypes

import concourse.bass as bass
import concourse.tile as tile
from concourse import bacc, mybir

N_CORES = 8
B = 4096
I = 512
O = 512
C = 10
BS = B // N_CORES          # batch rows per core (512)
W_SPACING = 2.0 / (C - 1)  # rbf width == center spacing
INV_W2 = 1.0 / (W_SPACING * W_SPACING)  # 20.25
NT = I // 128              # 4 i-tiles
NKT = NT * (C + 1)         # 44 k-tiles of 128 (10 centers + silu)

_CACHE = {}


def _build():
    """Build and finalize the per-core Bass module (same on all cores)."""
    nc = bacc.Bacc("TRN2", target_bir_lowering=False, debug=False,
                   num_devices=N_CORES)
    f32 = mybir.dt.float32
    bf16 = mybir.dt.bfloat16
    xt_d = nc.dram_tensor("xt", (I, BS), f32, kind="ExternalInput")
    w_d = nc.dram_tensor("w", (NKT, 128, O), bf16, kind="ExternalInput")
    out_d = nc.dram_tensor("out", (BS, O), f32, kind="ExternalOutput")

    centers = np.linspace(-1.0, 1.0, C)

    with tile.TileContext(nc) as tc:
        with (
            tc.tile_pool(name="big", bufs=1) as big,
            tc.tile_pool(name="wpool", bufs=1) as wpool,
            tc.tile_pool(name="psum", bufs=1, space="PSUM") as psum,
        ):
            # ---- load weights (streamed; matmuls start as tiles land) ----
            w_sb = wpool.tile([128, NKT, O], bf16)
            for kt in range(NKT):
                nc.sync.dma_start(out=w_sb[:, kt, :], in_=w_d[kt])

            # ---- load x^T and compute activations ----
            xt_sb = big.tile([128, NT * BS], f32, tag="xt")
            for t in range(NT):
                nc.sync.dma_start(out=xt_sb[:, t * BS:(t + 1) * BS],
                                  in_=xt_d[t * 128:(t + 1) * 128, :])
            xn = big.tile([128, NT * BS], f32, tag="xn")
            nc.scalar.activation(out=xn[:], in_=xt_sb[:],
                                 func=mybir.ActivationFunctionType.Tanh)
            silu_bf = big.tile([128, NT * BS], bf16, tag="silu")
            nc.scalar.activation(out=silu_bf[:], in_=xn[:],
                                 func=mybir.ActivationFunctionType.Silu)
            sq = big.tile([128, NT * BS], f32, tag="sq")
            nc.vector.tensor_mul(out=sq[:], in0=xn[:], in1=xn[:])
            u = big.tile([128, NT * BS], f32, tag="u")
            nc.scalar.activation(out=u[:], in_=sq[:],
                                 func=mybir.ActivationFunctionType.Exp,
                                 scale=-INV_W2)

            # ---- per-center Gaussians: g_j = u * exp(a_j*xn + b_j) ----
            bias_sb = big.tile([128, C], f32, tag="bias")
            for j in range(C):
                b_j = -centers[j] * centers[j] * INV_W2
                nc.vector.memset(bias_sb[:, j:j + 1], float(b_j))
            basis = []
            for j in range(C):
                a_j = 2.0 * centers[j] * INV_W2
                e_j = big.tile([128, NT * BS], f32, tag=f"e{j % 2}")
                nc.scalar.activation(out=e_j[:], in_=xn[:],
                                     func=mybir.ActivationFunctionType.Exp,
                                     bias=bias_sb[:, j:j + 1], scale=float(a_j))
                g_j = big.tile([128, NT * BS], bf16, tag=f"g{j}")
                nc.vector.tensor_mul(out=g_j[:], in0=u[:], in1=e_j[:])
                basis.append(g_j)
            basis.append(silu_bf)  # channel 10 = residual/silu

            # ---- the big matmul: out[b,o] += basis_k[b] * w[k,o] ----
            ps = [psum.tile([128, O], mybir.dt.float32, name=f"ps{bt}")
                  for bt in range(4)]
            for j in range(C + 1):
                for t in range(NT):
                    kt = 4 * j + t if j < C else 4 * C + t
                    for bt in range(4):
                        nc.tensor.matmul(
                            ps[bt],
                            basis[j][:, t * BS + bt * 128: t * BS + (bt + 1) * 128],
                            w_sb[:, kt, :],
                            start=(kt == 0),
                            stop=(kt == NKT - 1),
                        )

            # ---- copy out and store ----
            for bt in range(4):
                o_sb = big.tile([128, O], f32, tag=f"o{bt}")
                nc.scalar.copy(out=o_sb[:], in_=ps[bt][:])
                nc.sync.dma_start(out=out_d[bt * 128:(bt + 1) * 128, :],
                                  in_=o_sb[:])
    nc.finalize()
    return nc


def _prep_inputs(x, coef, scale_base, scale_sp):
    """Host-side shard + layout prep (cheap numpy reshapes/casts)."""
    x = np.asarray(x, dtype=np.float32)
    coef = np.asarray(coef, dtype=np.float32)
    scale_base = np.asarray(scale_base, dtype=np.float32)
    scale_sp = np.asarray(scale_sp, dtype=np.float32)

    # W[k, o]: k ordered as (center j, i_tile t) then 4 silu tiles.
    wf = coef * scale_sp.T[:, :, None]          # [I, O, C]
    wf = wf.reshape(NT, 128, O, C).transpose(3, 0, 1, 2)  # [C, NT, 128, O]
    w_all = np.concatenate(
        [wf.reshape(C * NT, 128, O), scale_base.T.reshape(NT, 128, O)], axis=0
    ).astype(ml_dtypes.bfloat16)                 # [NKT, 128, O]

    in_maps = []
    for k in range(N_CORES):
        xs = x[k * BS:(k + 1) * BS, :]           # [BS, I]
        in_maps.append({"xt": np.ascontiguousarray(xs.T), "w": w_all})
    return in_maps


def _run(in_maps, trace=False):
    if "antenv.axon_hooks" not in sys.modules:
        try:
            from trn_agent_boot.trn_boot import _ntff_profile_via_ctypes
            _hook = _ntff_profile_via_ctypes("/opt/axon/libaxon_pjrt.so")
            _mod = types.ModuleType("antenv.axon_hooks")
            _mod.get_axon_ntff_profile_hook = lambda: _hook
            sys.modules["antenv.axon_hooks"] = _mod
        except Exception:
            pass
    from concourse.bass_utils import run_bass_kernel_spmd

    if "nc" not in _CACHE:
        _CACHE["nc"] = _build()
    return run_bass_kernel_spmd(_CACHE["nc"], in_maps,
                                core_ids=list(range(N_CORES)), trace=trace)


def kernel(x, coef, scale_base, scale_sp):
    in_maps = _prep_inputs(x, coef, scale_base, scale_sp)
    res = _run(in_maps, trace=False)
    out = np.concatenate([res.results[k]["out"] for k in range(N_CORES)],
                         axis=0)
    return out.astype(np.float32)


# revision 3
# speedup vs baseline: 1.4431x; 1.4431x over previous
"""HeterogeneousKANLayer forward on 8 Trainium2 NeuronCores.

Math (reference):
  xn    = tanh(x)                                  [B, I]
  base  = silu(xn)                                 [B, I]
  basis = exp(-((xn - c_j)/w)^2), c_j evenly spaced on [-1,1], w = 2/(C-1)
  out[b,o] = sum_{i,c} basis[b,i,c]*coef[i,o,c]*scale_sp[o,i]
           + sum_i base[b,i]*scale_base[o,i]

Kernel strategy (data-parallel over batch, 8 cores x 512 rows):
  Everything is one [512b, 5632k] @ [5632k, 512o] matmul per core, where
  k = (center, i) channels plus one silu channel group. Host folds
  scale_sp into coef and appends scale_base^T as the last 4 k-tiles.
  Gaussian per center j is computed as u * exp(a_j*xn + b_j) with
  u = exp(-xn^2/w^2): one ACT exp + one DVE multiply per center
  (ACT ~25us, DVE ~25us, PE ~37.5us bf16 - PE-bound at the fp32r/bf16
  roofline for this shard).
"""

import sys
import types

import numpy as np
import ml_dtypes

import concourse.bass as bass
import concourse.tile as tile
from concourse import bacc, mybir

N_CORES = 8
B = 4096
I = 512
O = 512
C = 10
BS = B // N_CORES          # batch rows per core (512)
W_SPACING = 2.0 / (C - 1)  # rbf width == center spacing
INV_W2 = 1.0 / (W_SPACING * W_SPACING)  # 20.25
NT = I // 128              # 4 i-tiles
NKT = NT * (C + 1)         # 44 k-tiles of 128 (10 centers + silu)

_CACHE = {}


def _build():
    """Build and finalize the per-core Bass module (same on all cores)."""
    nc = bacc.Bacc("TRN2", target_bir_lowering=False, debug=False,
                   num_devices=N_CORES)
    f32 = mybir.dt.float32
    bf16 = mybir.dt.bfloat16
    xt_d = nc.dram_tensor("xt", (I, BS), f32, kind="ExternalInput")
    w_d = nc.dram_tensor("w", (NKT, 128, O), bf16, kind="ExternalInput")
    out_d = nc.dram_tensor("out", (BS, O), f32, kind="ExternalOutput")

    centers = np.linspace(-1.0, 1.0, C)

    with tile.TileContext(nc) as tc:
        with (
            tc.tile_pool(name="big", bufs=1) as big,
            tc.tile_pool(name="wpool", bufs=1) as wpool,
            tc.tile_pool(name="psum", bufs=1, space="PSUM") as psum,
        ):
            # ---- load x^T FIRST (it heads the critical path) ----
            xt_sb = big.tile([128, NT * BS], f32, tag="xt")
            for t in range(NT):
                nc.sync.dma_start(out=xt_sb[:, t * BS:(t + 1) * BS],
                                  in_=xt_d[t * 128:(t + 1) * 128, :])

            # ---- weights stream behind xt, in 4-ktile chunks ----
            w_sb = wpool.tile([128, NKT, O], bf16)
            for j in range(NKT // 4):
                nc.sync.dma_start(
                    out=w_sb[:, 4 * j:4 * (j + 1), :],
                    in_=w_d[4 * j:4 * (j + 1)].rearrange("k p o -> p k o"))

            bias_sb = big.tile([128, C], f32, tag="bias")
            for j in range(C):
                b_j = -centers[j] * centers[j] * INV_W2
                nc.vector.memset(bias_sb[:, j:j + 1], float(b_j))

            xn = big.tile([128, NT * BS], f32, tag="xn")
            nc.scalar.activation(out=xn[:], in_=xt_sb[:],
                                 func=mybir.ActivationFunctionType.Tanh)
            sq = big.tile([128, NT * BS], f32, tag="sq")
            nc.vector.tensor_mul(out=sq[:], in0=xn[:], in1=xn[:])

            # ---- per-center Gaussians: g_j = u * exp(a_j*xn + b_j) ----
            # ACT order: e_0, u, e_1..e_9, silu — shortest path to g_0,
            # silu last (its k-tiles are consumed last by the PE).
            e_tiles = []
            for j in range(2):
                a_j = 2.0 * centers[j] * INV_W2
                e_j = big.tile([128, NT * BS], f32, tag=f"e{j}")
                nc.scalar.activation(out=e_j[:], in_=xn[:],
                                     func=mybir.ActivationFunctionType.Exp,
                                     bias=bias_sb[:, j:j + 1], scale=float(a_j))
                e_tiles.append(e_j)
            u = big.tile([128, NT * BS], f32, tag="u")
            nc.scalar.activation(out=u[:], in_=sq[:],
                                 func=mybir.ActivationFunctionType.Exp,
                                 scale=-INV_W2)
            basis = []
            for j in range(C):
                if j >= 2:
                    a_j = 2.0 * centers[j] * INV_W2
                    e_j = big.tile([128, NT * BS], f32, tag=f"e{2 + (j % 2)}")
                    nc.scalar.activation(out=e_j[:], in_=xn[:],
                                         func=mybir.ActivationFunctionType.Exp,
                                         bias=bias_sb[:, j:j + 1],
                                         scale=float(a_j))
                else:
                    e_j = e_tiles[j]
                g_j = big.tile([128, NT * BS], bf16, tag=f"g{j}")
                nc.vector.tensor_mul(out=g_j[:], in0=u[:], in1=e_j[:])
                basis.append(g_j)
            silu_bf = big.tile([128, NT * BS], bf16, tag="silu")
            nc.scalar.activation(out=silu_bf[:], in_=xn[:],
                                 func=mybir.ActivationFunctionType.Silu)
            basis.append(silu_bf)  # channel 10 = residual/silu

            # ---- the big matmul: out[b,o] += basis_k[b] * w[k,o] ----
            ps = [psum.tile([128, O], mybir.dt.float32, name=f"ps{bt}")
                  for bt in range(4)]
            for j in range(C + 1):
                for t in range(NT):
                    kt = 4 * j + t if j < C else 4 * C + t
                    for bt in range(4):
                        nc.tensor.matmul(
                            ps[bt],
                            basis[j][:, t * BS + bt * 128: t * BS + (bt + 1) * 128],
                            w_sb[:, kt, :],
                            start=(kt == 0),
                            stop=(kt == NKT - 1),
                        )

            # ---- copy out and store ----
            for bt in range(4):
                o_sb = big.tile([128, O], f32, tag=f"o{bt}")
                nc.scalar.copy(out=o_sb[:], in_=ps[bt][:])
                nc.sync.dma_start(out=out_d[bt * 128:(bt + 1) * 128, :],
                                  in_=o_sb[:])
    nc.finalize()
    return nc


def _prep_inputs(x, coef, scale_base, scale_sp):
    """Host-side shard + layout prep (cheap numpy reshapes/casts)."""
    x = np.asarray(x, dtype=np.float32)
    coef = np.asarray(coef, dtype=np.float32)
    scale_base = np.asarray(scale_base, dtype=np.float32)
    scale_sp = np.asarray(scale_sp, dtype=np.float32)

    # W[k, o]: k ordered as (center j, i_tile t) then 4 silu tiles.
    wf = coef * scale_sp.T[:, :, None]          # [I, O, C]
    wf = wf.reshape(NT, 128, O, C).transpose(3, 0, 1, 2)  # [C, NT, 128, O]
    w_all = np.concatenate(
        [wf.reshape(C * NT, 128, O), scale_base.T.reshape(NT, 128, O)], axis=0
    ).astype(ml_dtypes.bfloat16)                 # [NKT, 128, O]

    in_maps = []
    for k in range(N_CORES):
        xs = x[k * BS:(k + 1) * BS, :]           # [BS, I]
        in_maps.append({"xt": np.ascontiguousarray(xs.T), "w": w_all})
    return in_maps


def _run(in_maps, trace=False):
    if "antenv.axon_hooks" not in sys.modules:
        try:
            from trn_agent_boot.trn_boot import _ntff_profile_via_ctypes
            _hook = _ntff_profile_via_ctypes("/opt/axon/libaxon_pjrt.so")
            _mod = types.ModuleType("antenv.axon_hooks")
            _mod.get_axon_ntff_profile_hook = lambda: _hook
            sys.modules["antenv.axon_hooks"] = _mod
        except Exception:
            pass
    from concourse.bass_utils import run_bass_kernel_spmd

    if "nc" not in _CACHE:
        _CACHE["nc"] = _build()
    return run_bass_kernel_spmd(_CACHE["nc"], in_maps,
                                core_ids=list(range(N_CORES)), trace=trace)


def kernel(x, coef, scale_base, scale_sp):
    in_maps = _prep_inputs(x, coef, scale_base, scale_sp)
    res = _run(in_maps, trace=False)
    out = np.concatenate([res.results[k]["out"] for k in range(N_CORES)],
                         axis=0)
    return out.astype(np.float32)


# revision 5
# speedup vs baseline: 1.5347x; 1.0635x over previous
"""HeterogeneousKANLayer forward on 8 Trainium2 NeuronCores.

Math (reference):
  xn    = tanh(x)                                  [B, I]
  base  = silu(xn)                                 [B, I]
  basis = exp(-((xn - c_j)/w)^2), c_j evenly spaced on [-1,1], w = 2/(C-1)
  out[b,o] = sum_{i,c} basis[b,i,c]*coef[i,o,c]*scale_sp[o,i]
           + sum_i base[b,i]*scale_base[o,i]

Kernel strategy (data-parallel over batch, 8 cores x 512 rows):
  Everything is one [512b, 5632k] @ [5632k, 512o] matmul per core, where
  k = (center, i) channels plus one silu channel group. Host folds
  scale_sp into coef and appends scale_base^T as the last 4 k-tiles.
  Gaussian per center j is computed as u * exp(a_j*xn + b_j) with
  u = exp(-xn^2/w^2): one ACT exp + one DVE multiply per center
  (ACT ~25us, DVE ~25us, PE ~37.5us bf16 - PE-bound at the fp32r/bf16
  roofline for this shard).
"""

import sys
import types

import numpy as np
import ml_dtypes

import concourse.bass as bass
import concourse.tile as tile
from concourse import bacc, mybir

N_CORES = 8
B = 4096
I = 512
O = 512
C = 10
BS = B // N_CORES          # batch rows per core (512)
W_SPACING = 2.0 / (C - 1)  # rbf width == center spacing
INV_W2 = 1.0 / (W_SPACING * W_SPACING)  # 20.25
NT = I // 128              # 4 i-tiles
NKT = NT * (C + 1)         # 44 k-tiles of 128 (10 centers + silu)

_CACHE = {}


def _build():
    """Build and finalize the per-core Bass module (same on all cores)."""
    nc = bacc.Bacc("TRN2", target_bir_lowering=False, debug=False,
                   num_devices=N_CORES)
    f32 = mybir.dt.float32
    bf16 = mybir.dt.bfloat16
    xt_d = nc.dram_tensor("xt", (I, BS), f32, kind="ExternalInput")
    w_d = nc.dram_tensor("w", (NKT, 128, O), bf16, kind="ExternalInput")
    out_d = nc.dram_tensor("out", (BS, O), f32, kind="ExternalOutput")

    centers = np.linspace(-1.0, 1.0, C)

    with tile.TileContext(nc) as tc:
        with (
            tc.tile_pool(name="big", bufs=1) as big,
            tc.tile_pool(name="wpool", bufs=1) as wpool,
            tc.tile_pool(name="psum", bufs=1, space="PSUM") as psum,
        ):
            # ---- warm-ups: pull ACT table loads + PE HAM ramp off the
            # critical path (tiny ops on a scratch tile at t~0) ----
            warm = big.tile([128, 8], f32, tag="warm")
            nc.vector.memset(warm[:], 0.0)
            for fn in (mybir.ActivationFunctionType.Tanh,
                       mybir.ActivationFunctionType.Exp,
                       mybir.ActivationFunctionType.Silu):
                nc.scalar.activation(out=warm[:], in_=warm[:], func=fn)
            warm_w = wpool.tile([128, O], bf16, tag="warmw")
            nc.vector.memset(warm_w[:], 0.0)
            ps_warm = psum.tile([128, O], mybir.dt.float32, name="pswarm")
            for _ in range(40):
                nc.tensor.matmul(ps_warm, warm_w[:, :128], warm_w[:],
                                 start=True, stop=True)

            # ---- load x^T FIRST (it heads the critical path) ----
            xt_sb = big.tile([128, NT * BS], f32, tag="xt")
            for t in range(NT):
                nc.sync.dma_start(out=xt_sb[:, t * BS:(t + 1) * BS],
                                  in_=xt_d[t * 128:(t + 1) * 128, :])

            # ---- weights stream behind xt, in 4-ktile chunks ----
            w_sb = wpool.tile([128, NKT, O], bf16)
            for j in range(NKT // 4):
                nc.sync.dma_start(
                    out=w_sb[:, 4 * j:4 * (j + 1), :],
                    in_=w_d[4 * j:4 * (j + 1)].rearrange("k p o -> p k o"))

            bias_sb = big.tile([128, C], f32, tag="bias")
            for j in range(C):
                b_j = -centers[j] * centers[j] * INV_W2
                nc.vector.memset(bias_sb[:, j:j + 1], float(b_j))

            # ---- g_0 path chunked at [128,512] so the PE can start on
            # (c=0, t=0) as soon as possible ----
            def sl(t):
                return slice(t * BS, (t + 1) * BS)

            a0 = 2.0 * centers[0] * INV_W2
            xn = big.tile([128, NT * BS], f32, tag="xn")
            sq = big.tile([128, NT * BS], f32, tag="sq")
            e_0 = big.tile([128, NT * BS], f32, tag="e0")
            u = big.tile([128, NT * BS], f32, tag="u")
            g_0 = big.tile([128, NT * BS], bf16, tag="g0")
            for t in range(NT):
                nc.scalar.activation(out=xn[:, sl(t)], in_=xt_sb[:, sl(t)],
                                     func=mybir.ActivationFunctionType.Tanh)
                nc.vector.tensor_mul(out=sq[:, sl(t)], in0=xn[:, sl(t)],
                                     in1=xn[:, sl(t)])
                nc.scalar.activation(out=e_0[:, sl(t)], in_=xn[:, sl(t)],
                                     func=mybir.ActivationFunctionType.Exp,
                                     bias=bias_sb[:, 0:1], scale=float(a0))
                nc.scalar.activation(out=u[:, sl(t)], in_=sq[:, sl(t)],
                                     func=mybir.ActivationFunctionType.Exp,
                                     scale=-INV_W2)
                nc.vector.tensor_mul(out=g_0[:, sl(t)], in0=u[:, sl(t)],
                                     in1=e_0[:, sl(t)])

            # ---- remaining centers full-tile; silu last (consumed last) ----
            basis = [g_0]
            for j in range(1, C):
                a_j = 2.0 * centers[j] * INV_W2
                e_j = big.tile([128, NT * BS], f32, tag=f"e{1 + (j % 2)}")
                nc.scalar.activation(out=e_j[:], in_=xn[:],
                                     func=mybir.ActivationFunctionType.Exp,
                                     bias=bias_sb[:, j:j + 1], scale=float(a_j))
                g_j = big.tile([128, NT * BS], bf16, tag=f"g{j}")
                nc.vector.tensor_mul(out=g_j[:], in0=u[:], in1=e_j[:])
                basis.append(g_j)
            silu_bf = big.tile([128, NT * BS], bf16, tag="silu")
            nc.scalar.activation(out=silu_bf[:], in_=xn[:],
                                 func=mybir.ActivationFunctionType.Silu)
            basis.append(silu_bf)  # channel 10 = residual/silu

            # ---- the big matmul: out[b,o] += basis_k[b] * w[k,o] ----
            ps = [psum.tile([128, O], mybir.dt.float32, name=f"ps{bt}")
                  for bt in range(4)]
            for j in range(C + 1):
                for t in range(NT):
                    kt = 4 * j + t if j < C else 4 * C + t
                    for bt in range(4):
                        nc.tensor.matmul(
                            ps[bt],
                            basis[j][:, t * BS + bt * 128: t * BS + (bt + 1) * 128],
                            w_sb[:, kt, :],
                            start=(kt == 0),
                            stop=(kt == NKT - 1),
                        )

            # ---- copy out and store ----
            for bt in range(4):
                o_sb = big.tile([128, O], f32, tag=f"o{bt}")
                nc.scalar.copy(out=o_sb[:], in_=ps[bt][:])
                nc.sync.dma_start(out=out_d[bt * 128:(bt + 1) * 128, :],
                                  in_=o_sb[:])
    nc.finalize()
    return nc


def _prep_inputs(x, coef, scale_base, scale_sp):
    """Host-side shard + layout prep (cheap numpy reshapes/casts)."""
    x = np.asarray(x, dtype=np.float32)
    coef = np.asarray(coef, dtype=np.float32)
    scale_base = np.asarray(scale_base, dtype=np.float32)
    scale_sp = np.asarray(scale_sp, dtype=np.float32)

    # W[k, o]: k ordered as (center j, i_tile t) then 4 silu tiles.
    wf = coef * scale_sp.T[:, :, None]          # [I, O, C]
    wf = wf.reshape(NT, 128, O, C).transpose(3, 0, 1, 2)  # [C, NT, 128, O]
    w_all = np.concatenate(
        [wf.reshape(C * NT, 128, O), scale_base.T.reshape(NT, 128, O)], axis=0
    ).astype(ml_dtypes.bfloat16)                 # [NKT, 128, O]

    in_maps = []
    for k in range(N_CORES):
        xs = x[k * BS:(k + 1) * BS, :]           # [BS, I]
        in_maps.append({"xt": np.ascontiguousarray(xs.T), "w": w_all})
    return in_maps


def _run(in_maps, trace=False):
    if "antenv.axon_hooks" not in sys.modules:
        try:
            from trn_agent_boot.trn_boot import _ntff_profile_via_ctypes
            _hook = _ntff_profile_via_ctypes("/opt/axon/libaxon_pjrt.so")
            _mod = types.ModuleType("antenv.axon_hooks")
            _mod.get_axon_ntff_profile_hook = lambda: _hook
            sys.modules["antenv.axon_hooks"] = _mod
        except Exception:
            pass
    from concourse.bass_utils import run_bass_kernel_spmd

    if "nc" not in _CACHE:
        _CACHE["nc"] = _build()
    return run_bass_kernel_spmd(_CACHE["nc"], in_maps,
                                core_ids=list(range(N_CORES)), trace=trace)


def kernel(x, coef, scale_base, scale_sp):
    in_maps = _prep_inputs(x, coef, scale_base, scale_sp)
    res = _run(in_maps, trace=False)
    out = np.concatenate([res.results[k]["out"] for k in range(N_CORES)],
                         axis=0)
    return out.astype(np.float32)
